# revision 40
# baseline (speedup 1.0000x reference)
"""BERT self-attention Bass/Tile kernel for 8 Trainium2 NeuronCores.

Problem: hidden [2, 2048, 768], 12 heads x 64 dim, additive mask [2,1,1,2048].
Sharding: batch x head-group. Core c handles batch b = c // 4 and global heads
3*(c%4) .. 3*(c%4)+2 (columns 192*(c%4) .. +192 of Wq/Wk/Wv).  Each core
computes its 3 heads' full attention locally; outputs are concatenated on the
host (no cross-device communication).

Per-core pipeline (one TileContext):
  X [2048,768] --gpsimd cast fp16, PE transpose--> X_T [768, 2048] (fp16)
  Q_T/K_T = W.T @ X_T  (heads 0,1 packed M=128; head2's Q+K col-tiled M=64)
  V[k, d] computed DIRECTLY via lhsT=X_T chunk (no V transpose):
      V[kc] layout [V_h0|e|V_h1|e|V_h2|e], e = exp(mask_k) (1.0 when mask==0)
  scores_T[k, q] = K_T.T @ Q_T, N=512 per matmul; two streams row-tiled
      (prow 0 / 64) write the two 512-col halves of one [128,1024] PSUM tile
  probs = exp(scores/8) fp16:
      ~75% of tiles on ScalarE (activation Exp, scale=0.125)
      ~25% on VectorE via a Schraudolph bit-trick: i16 = round(A*s + B)
      bitcast to fp16 (~1.8% rms rel err; denominator from the same probs
      cancels the common mode, final rel err ~1e-2 worst case)
  ctx[q, 65] accumulation per (head, q-subtile): 16 matmuls N=65, all three
      heads sharing one PSUM bank (cols 0:65|65:130|130:195)
  normalize: one reciprocal [128,3] + one scalar_tensor_tensor with a
      stride-0 broadcast of the reciprocals -> out tile -> DMA

The additive mask is folded into V: exp(s + m_k) = exp(s) * exp(m_k); both
numerator and denominator columns are pre-scaled by exp(m_k).  Zero-mask and
zero-bias (the common case) compile specialized variants.
"""

import numpy as np

import concourse.bass as bass
import concourse.tile as tile
from concourse import bacc, mybir
from concourse.bass_utils import run_bass_kernel_spmd
from concourse.masks import make_identity

F32 = mybir.dt.float32
F16 = mybir.dt.float16
I16 = mybir.dt.int16
EXP = mybir.ActivationFunctionType.Exp
MULT = mybir.AluOpType.mult
ADD = mybir.AluOpType.add

S = 2048           # sequence length
DM = 768           # model dim
DH = 64            # head dim
FC = DM // 128     # 6 f-chunks (contraction for projections)
KC = S // 128      # 16 k-chunks
QB = 512           # q block width (one N=512 matmul per k-chunk per stream)

# Schraudolph fp16 exp bits: i16 = round(184.66496*s + 15301) ~ fp16(exp(s/8))
SCHR_A = 184.66496
SCHR_B = 15360.0 - 59.0
DVE_EXP = True
DEBUG = False

# Score units: A-units pair heads 0/1 on one q-block J and k-chunk t;
# C-units pair head2 with itself on (kc=2t, kc=2t+1) of q-block J.  Units are
# ordered J-major, interleaved (A 2i, A 2i+1, C i) so that the ctx chains of
# q-block J can stream kc-by-kc right behind score production.


def _build_kernel(zero_mask: bool, zero_bias: bool) -> bass.Bass:
    nc = bacc.Bacc()

    x_d = nc.declare_dram_parameter("x", [S, DM], F32, isOutput=False)
    wq_d = nc.declare_dram_parameter("wq", [DM, 192], F32, isOutput=False)
    wk_d = nc.declare_dram_parameter("wk", [DM, 192], F32, isOutput=False)
    wv_d = nc.declare_dram_parameter("wv", [DM, 192], F32, isOutput=False)
    bq_d = nc.declare_dram_parameter("bq", [192], F32, isOutput=False)
    bk_d = nc.declare_dram_parameter("bk", [192], F32, isOutput=False)
    bv_d = nc.declare_dram_parameter("bv", [192], F32, isOutput=False)
    m_d = nc.declare_dram_parameter("mask", [S], F32, isOutput=False)
    out_d = nc.declare_dram_parameter("out", [S, 192], F32, isOutput=True)
    dbg = {}
    if DEBUG:
        for nm, shape in [("d_xt0", [128, S]), ("d_qt2", [128, S]),
                          ("d_kt2", [128, S]), ("d_qts", [128, S]),
                          ("d_kts", [128, S]), ("d_v0", [128, 195]),
                          ("d_pa", [128, 1024]), ("d_pc", [128, 1024]),
                          ("d_pc_all", [128, 8192]), ("d_v_all", [128, 3120])]:
            dbg[nm] = nc.declare_dram_parameter(nm, shape, F16, isOutput=True)

    with tile.TileContext(nc) as tc:
        _attention(tc, x_d, (wq_d, wk_d, wv_d), (bq_d, bk_d, bv_d), m_d, out_d,
                   zero_mask, zero_bias, dbg)
    nc.compile()
    return nc


def _attention(tc, x_d, w_ds, b_ds, m_d, out_d, zero_mask, zero_bias, dbg={}):
    nc = tc.nc

    def dbg_dump(name, src):
        if name in dbg:
            nc.sync.dma_start(out=dbg[name][:, :], in_=src)

    const = tc.alloc_tile_pool(name="const", bufs=1)
    xpool = tc.alloc_tile_pool(name="xpool", bufs=5)
    persist = tc.alloc_tile_pool(name="persist", bufs=1)
    probs_pool = tc.alloc_tile_pool(name="probs", bufs=44)
    small = tc.alloc_tile_pool(name="small", bufs=4)
    outp = tc.alloc_tile_pool(name="outp", bufs=1)
    ps = tc.alloc_tile_pool(name="ps", bufs=2, space="PSUM")

    # The m=0 X DMAs are issued first so the SP queue starts the big loads
    # before anything else; they overlap all of the setup below.
    x32_tiles = {}

    def load_x_dma(m):
        for j in range(4):
            qt = 4 * m + j
            xt32 = xpool.tile([128, DM], F32, name=f"x_{qt}", tag="x", bufs=8)
            nc.sync.dma_start(out=xt32, in_=x_d[128 * qt:128 * (qt + 1), :])
            x32_tiles[qt] = xt32

    load_x_dma(0)

    # --- constants: mask, identity, weights, biases -------------------------
    mask_t = const.tile([128, KC], F32)  # mask[128*i + p] at [p, i]
    nc.gpsimd.dma_start(out=mask_t, in_=m_d[:].rearrange("(i p) -> p i", p=128))
    expm = const.tile([128, KC], F32)    # exp(mask); also preloads exp table
    nc.scalar.activation(expm, mask_t, EXP)

    ident16 = const.tile([128, 128], F16)
    make_identity(nc, ident16)

    # PE warmup: ~3.5us of junk matmuls flips the HAM clock gate to 2.4 GHz
    # while the first X DMAs land.
    warm = ps.tile([128, 512], F32, name="warm", tag="sc", bufs=2)
    for i in range(30):
        nc.tensor.matmul(warm[:, 0:128], ident16, ident16,
                         start=True, stop=True)

    w16 = {}   # (t, f) -> [128, 192] fp16
    for t in range(3):
        for f in range(FC):
            w32 = small.tile([128, 192], F32, name=f"w32_{t}_{f}", tag="w32",
                             bufs=9)
            nc.gpsimd.dma_start(out=w32, in_=w_ds[t][128 * f:128 * (f + 1), :])
            wt = const.tile([128, 192], F16, name=f"w16_{t}_{f}")
            nc.vector.tensor_copy(out=wt, in_=w32)
            w16[(t, f)] = wt

    bias_pair = []   # [128,1] heads 0,1 of each of q,k
    bias_solo = []   # [128,1] head2, duplicated into both halves
    bvb = None       # [128,192] broadcast of bv (added to the output)
    if not zero_bias:
        for t in range(2):
            bp = const.tile([128, 1], F32, name=f"bias_pair_{t}")
            nc.gpsimd.dma_start(
                out=bp, in_=b_ds[t][0:128].rearrange("(p o) -> p o", o=1))
            bias_pair.append(bp)
            bs = const.tile([128, 1], F32, name=f"bias_solo_{t}")
            nc.gpsimd.dma_start(
                out=bs[0:64], in_=b_ds[t][128:192].rearrange("(p o) -> p o", o=1))
            nc.gpsimd.dma_start(
                out=bs[64:128], in_=b_ds[t][128:192].rearrange("(p o) -> p o", o=1))
            bias_solo.append(bs)
        # bvb[p, d] = bv[d] via a K=1 matmul broadcast: ones[1,128].T @ bv[1,192]
        ones_row = const.tile([1, 128], F32, name="ones_row")
        nc.gpsimd.memset(ones_row, 1.0)
        bv_row = const.tile([1, 192], F32, name="bv_row")
        nc.gpsimd.dma_start(out=bv_row, in_=b_ds[2][:].rearrange("(o d) -> o d", o=1))
        bvp = ps.tile([128, 512], F32, name="bvp", tag="sm", bufs=2)
        nc.tensor.matmul(bvp[:, 0:192], ones_row, bv_row, start=True, stop=True)
        bvb = const.tile([128, 192], F32, name="bvb")
        nc.vector.tensor_copy(out=bvb, in_=bvp[:, 0:192])

    # --- persistent projection outputs --------------------------------------
    XT = [persist.tile([128, S], F16, name=f"XT_{f}") for f in range(FC)]
    QT2 = persist.tile([128, S], F16)   # rows 0:64 head0, 64:128 head1
    KT2 = persist.tile([128, S], F16)
    QTs = persist.tile([128, S], F16)   # head2 duplicated into both halves
    KTs = persist.tile([128, S], F16)
    # V[kc] layout: [V_h0(64) | e | V_h1(64) | e | V_h2(64) | e]
    V = [persist.tile([128, 195], F16, name=f"V_{kc}") for kc in range(KC)]

    out_tiles = [outp.tile([128, 192], F32, name=f"o_{u}") for u in range(16)]

    def load_x_block(m):
        """Cast fp16 on gpsimd, PE-transpose into XT."""
        x16s = []
        for j in range(4):
            qt = 4 * m + j
            xt32 = x32_tiles.pop(qt)
            x16 = xpool.tile([128, DM], F16, name=f"x16_{qt}", tag="x16")
            nc.gpsimd.tensor_copy(out=x16, in_=xt32)
            x16s.append(x16)
        for f in range(FC):
            tp = ps.tile([128, 512], F16, name=f"xt_ps_{m}_{f}", tag="sm",
                         bufs=2)
            for j in range(4):
                nc.tensor.transpose(
                    tp[:, 128 * j:128 * (j + 1)],
                    x16s[j][:, 128 * f:128 * (f + 1)],
                    ident16,
                )
            nc.vector.tensor_copy(out=XT[f][:, 512 * m:512 * (m + 1)], in_=tp)

    def proj_pair(t, dst_pair, m):
        """Heads 0,1 of q (t=0) or k (t=1): M=128 weight, N=512 stream."""
        cols = slice(512 * m, 512 * (m + 1))
        pp = ps.tile([128, 512], F32, name=f"proj_{t}_{m}_p", tag="sm", bufs=2)
        for f in range(FC):
            nc.tensor.matmul(pp, w16[(t, f)][:, 0:128], XT[f][:, cols],
                             start=(f == 0), stop=(f == FC - 1))
        if zero_bias:
            nc.vector.tensor_copy(out=dst_pair[:, cols], in_=pp)
        else:
            nc.vector.tensor_scalar_add(out=dst_pair[:, cols], in0=pp,
                                        scalar1=bias_pair[t])

    def proj_qk2(m):
        """Head2's Q and K, col-tiled M=64 chains sharing one PSUM bank.

        Q lands in rows qrow:qrow+64 (qrow = 0 for even m, 64 for odd m) so
        QTs gets each q-block at the row parity the paired score streams
        expect; K lands in the other half and is duplicated into both halves
        of KTs (and QTs' other half) by SBUF->SBUF DMA."""
        cols = slice(512 * m, 512 * (m + 1))
        qrow = 0 if m % 2 == 0 else 64
        krow = 64 - qrow
        sp = ps.tile([128, 512], F32, name=f"proj_s_{m}", tag="sm", bufs=2)
        prev = None
        for f in range(FC):
            ma = nc.tensor.matmul(sp[qrow:qrow + 64], w16[(0, f)][:, 128:192],
                                  XT[f][:, cols],
                                  start=(f == 0), stop=(f == FC - 1))
            if prev is not None:
                tile.add_dep_helper(ma.ins, prev.ins, sync=False,
                                    reason="qk2 col-pair order")
            mb = nc.tensor.matmul(sp[krow:krow + 64], w16[(1, f)][:, 128:192],
                                  XT[f][:, cols],
                                  start=(f == 0), stop=(f == FC - 1))
            tile.add_dep_helper(mb.ins, ma.ins, sync=False,
                                reason="qk2 col-pair order")
            prev = mb
        if zero_bias:
            nc.vector.tensor_copy(out=QTs[qrow:qrow + 64, cols],
                                  in_=sp[qrow:qrow + 64])
            nc.vector.tensor_copy(out=KTs[krow:krow + 64, cols],
                                  in_=sp[krow:krow + 64])
        else:
            nc.vector.tensor_scalar_add(out=QTs[qrow:qrow + 64, cols],
                                        in0=sp[qrow:qrow + 64],
                                        scalar1=bias_solo[0][qrow:qrow + 64])
            nc.vector.tensor_scalar_add(out=KTs[krow:krow + 64, cols],
                                        in0=sp[krow:krow + 64],
                                        scalar1=bias_solo[1][krow:krow + 64])
        # duplicate head2 q/k into the other partition half for row tiling
        nc.gpsimd.dma_start(out=QTs[krow:krow + 64, cols],
                            in_=QTs[qrow:qrow + 64, cols])
        nc.gpsimd.dma_start(out=KTs[qrow:qrow + 64, cols],
                            in_=KTs[krow:krow + 64, cols])

    def v_direct(kc):
        """V[kc][k, d] = X[k,:] @ Wv directly: lhsT = X_T chunk (FWL'd fp16
        weights), rhs = Wv chunk, N=192.  No transpose, one strided copy."""
        vp = ps.tile([128, 512], F32, name=f"v_ps_{kc}", tag="sm", bufs=2)
        for f in range(FC):
            nc.tensor.matmul(vp[:, 0:192],
                             XT[f][:, 128 * kc:128 * (kc + 1)],
                             w16[(2, f)],
                             start=(f == 0), stop=(f == FC - 1))
        vdst = bass.AP(tensor=V[kc].tensor, offset=V[kc].offset,
                       ap=[V[kc].ap[0], [65, 3], [1, 64]])
        vsrc = bass.AP(tensor=vp.tensor, offset=vp.offset,
                       ap=[vp.ap[0], [64, 3], [1, 64]])
        ecol = bass.AP(tensor=V[kc].tensor, offset=V[kc].offset + 64,
                       ap=[V[kc].ap[0], [65, 3]])
        if zero_mask and zero_bias:
            nc.vector.tensor_copy(out=vdst, in_=vsrc)
        elif zero_mask:
            # bias folds into the output (ctx = ctx_nobias + bv); plain copy
            nc.vector.tensor_copy(out=vdst, in_=vsrc)
        else:
            # (v [+ bv]) * exp(m_k); bv also shifts ctx but the mask-weighted
            # denominator no longer cancels it, so scale V directly.
            tmp = small.tile([128, 192], F32, name=f"vt_{kc}", tag="vtmp")
            if zero_bias:
                nc.vector.tensor_copy(out=tmp, in_=vp[:, 0:192])
            else:
                nc.vector.tensor_tensor(out=tmp, in0=vp[:, 0:192], in1=bvb,
                                        op=ADD)
            tsrc = bass.AP(tensor=tmp.tensor, offset=tmp.offset,
                           ap=[tmp.ap[0], [64, 3], [1, 64]])
            nc.vector.tensor_scalar_mul(out=vdst, in0=tsrc,
                                        scalar1=expm[:, kc:kc + 1])
        if zero_mask:
            nc.gpsimd.memset(ecol, 1.0)
        else:
            esrc = bass.AP(tensor=expm.tensor, offset=expm.offset + kc,
                           ap=[expm.ap[0], [0, 3]])
            nc.vector.tensor_copy(out=ecol, in_=esrc)

    # --- scores + exp --------------------------------------------------------
    # unit = (kind, J, t): one [128,1024] PSUM tile, two N=512 matmuls on
    # disjoint row halves, one exp (ScalarE or VectorE-Schraudolph).
    # A-unit: streams (h0,J,prow0)+(h1,J,prow64), both on kc=t.
    # C-unit: stream (h2,J,prow0) on kc=2t, (h2,J,prow64) on 2t+1.
    probs_store = {}   # (head, J, kc) -> (tile, col_offset)

    def emit_unit(kind, J, t):
        sc = ps.tile([128, 1024], F32, name=f"sc_{kind}_{J}_{t}", tag="sc",
                     bufs=2)
        if kind == "A":
            streams = [(0, J, 0, t), (1, J, 64, t)]
        else:
            streams = [(2, J, 0, 2 * t), (2, J, 64, 2 * t + 1)]
        prev = None
        for i, (h, Jq, prow, kc) in enumerate(streams):
            KT = KT2 if h < 2 else KTs
            QT = QT2 if h < 2 else QTs
            mm = nc.tensor.matmul(
                sc[:, 512 * i:512 * (i + 1)],
                KT[prow:prow + 64, 128 * kc:128 * (kc + 1)],
                QT[prow:prow + 64, 512 * Jq:512 * (Jq + 1)],
                start=True, stop=True)
            if prev is not None:
                tile.add_dep_helper(mm.ins, prev.ins, sync=False,
                                    reason="score pair adjacency")
            prev = mm
        pt = probs_pool.tile([128, 1024], F16, name=f"pb_{kind}_{J}_{t}",
                             tag="probs")
        if DVE_EXP and t % 4 == 1:
            # VectorE Schraudolph: i16 bits of fp16(exp(s/8))
            nc.vector.tensor_scalar(
                pt[:, 0:1024].bitcast(I16), sc[:, 0:1024],
                SCHR_A, SCHR_B, MULT, ADD)
        else:
            nc.scalar.activation(pt, sc, EXP, scale=0.125)
        for i, (h, Jq, prow, kc) in enumerate(streams):
            probs_store[(h, Jq, kc)] = (pt, 512 * i)
        if kind == "A":
            a_cnt[J] += 1
        else:
            c_cnt[J] += 1
        if J == 0 and t == 0:
            dbg_dump("d_pa" if kind == "A" else "d_pc", pt)
        if "d_pc_all" in dbg and kind == "C" and J == 0:
            nc.sync.dma_start(out=dbg["d_pc_all"][:, 1024 * t:1024 * (t + 1)],
                              in_=pt)

    # --- ctx chains ----------------------------------------------------------
    # The 4 q-subtiles of q-block J accumulate concurrently, two per PSUM
    # bank: cxA holds u=4J+0 (cols 0:195) and u=4J+1 (cols 256:451), cxB the
    # other two.  Only the head J-block paces (strict FIFO) so the two cx
    # ring slots can never be claimed out of order.  Chain matmuls stream
    # kc-by-kc as soon as all three heads' probs for that kc exist.
    a_cnt = [0] * 4
    c_cnt = [0] * 4
    v_cnt = [0]        # v_direct(kc) emitted for kc < v_cnt[0]

    class JBlock:
        def __init__(self, J):
            self.J = J
            self.kc = 0
            self.cx = None     # [cxA, cxB]
            self.prev = [None, None]
            self.done = False

        def avail(self):
            return min(a_cnt[self.J], 2 * c_cnt[self.J], v_cnt[0])

        def pace(self):
            J = self.J
            hi = self.avail()
            if hi <= self.kc:
                return
            if self.cx is None:
                self.cx = [
                    ps.tile([128, 512], F32, name=f"cxA_{J}", tag="cx",
                            bufs=2),
                    ps.tile([128, 512], F32, name=f"cxB_{J}", tag="cx",
                            bufs=2),
                ]
            for kc in range(self.kc, hi):
                for pair in range(2):
                    cx = self.cx[pair]
                    for du in range(2):
                        s = 2 * pair + du
                        base = 256 * du
                        for h in range(3):
                            pt, off = probs_store[(h, J, kc)]
                            mm = nc.tensor.matmul(
                                cx[:, base + 65 * h:base + 65 * h + 65],
                                pt[:, off + 128 * s:off + 128 * (s + 1)],
                                V[kc][:, 65 * h:65 * h + 65],
                                start=(kc == 0 and du == 0 and h == 0),
                                stop=(kc == KC - 1))
                            if self.prev[pair] is not None:
                                tile.add_dep_helper(
                                    mm.ins, self.prev[pair].ins, sync=False,
                                    reason="cx chain order")
                            self.prev[pair] = mm
            self.kc = hi
            if self.kc == KC:
                for pair in range(2):
                    for du in range(2):
                        self._finish_u(4 * self.J + 2 * pair + du,
                                       self.cx[pair], 256 * du)
                self.done = True

        def _finish_u(self, u, cx, base):
            r = small.tile([128, 3], F32, name=f"r_{u}", tag="recip")
            den = bass.AP(tensor=cx.tensor, offset=cx.offset + base + 64,
                          ap=[cx.ap[0], [65, 3]])
            nc.vector.reciprocal(r, den)
            for h in range(3):
                nc.vector.tensor_scalar_mul(
                    out=out_tiles[u][:, 64 * h:64 * (h + 1)],
                    in0=cx[:, base + 65 * h:base + 65 * h + 64],
                    scalar1=r[:, h:h + 1])
            if not zero_bias and zero_mask:
                nc.vector.tensor_tensor(out=out_tiles[u], in0=out_tiles[u],
                                        in1=bvb, op=ADD)
            nc.sync.dma_start(out=out_d[128 * u:128 * (u + 1), :],
                              in_=out_tiles[u])

    # --- emission ------------------------------------------------------------
    units = []
    for J in range(4):
        for i in range(8):
            units.append(("A", J, 2 * i))
            units.append(("A", J, 2 * i + 1))
            units.append(("C", J, i))
    emitted = set()
    jqueue = [JBlock(J) for J in range(4)]

    def pace():
        while jqueue:
            jb = jqueue[0]
            jb.pace()
            if jb.done:
                jqueue.pop(0)
                continue
            break

    def try_emit(q_m, k_m, c_m, budget):
        # A-units: QT2 q-block J from proj_pair(Q, J), KT2 kc from
        # proj_pair(K, kc//4).  C-units: QTs/KTs both come from proj_qk2.
        for (kind, J, t) in units:
            if budget <= 0:
                return
            if (kind, J, t) in emitted:
                continue
            kmax = t if kind == "A" else 2 * t + 1
            if kind == "A":
                ok = J <= q_m and kmax // 4 <= k_m
            else:
                ok = J <= c_m and kmax // 4 <= c_m
            if ok:
                emit_unit(kind, J, t)
                emitted.add((kind, J, t))
                pace()
                budget -= 1

    for m in range(4):
        if m < 3:
            load_x_dma(m + 1)
        load_x_block(m)
        proj_pair(1, KT2, m)   # K first: scores need K columns
        try_emit(m - 1, m - 1, m - 1, 3)
        proj_pair(0, QT2, m)
        try_emit(m - 1, m, m - 1, 3)
        proj_qk2(m)
        try_emit(m, m, m, 4)
        for kc in range(4 * m, 4 * m + 4):
            v_direct(kc)
            v_cnt[0] = kc + 1
            try_emit(m, m, m, 2)

    if "d_v_all" in dbg:
        for kc in range(KC):
            nc.sync.dma_start(out=dbg["d_v_all"][:, 195 * kc:195 * (kc + 1)],
                              in_=V[kc])
    dbg_dump("d_xt0", XT[0])
    dbg_dump("d_qt2", QT2)
    dbg_dump("d_kt2", KT2)
    dbg_dump("d_qts", QTs)
    dbg_dump("d_kts", KTs)
    dbg_dump("d_v0", V[0])

    # steady state: remaining units interleaved with ctx chains
    for (kind, J, t) in units:
        if (kind, J, t) in emitted:
            continue
        emit_unit(kind, J, t)
        emitted.add((kind, J, t))
        pace()
    pace()

    for p in (ps, outp, small, probs_pool, persist, xpool, const):
        p.release()


_NC_CACHE = {}


def _get_nc(zero_mask: bool, zero_bias: bool):
    key = (zero_mask, zero_bias)
    if key not in _NC_CACHE:
        _NC_CACHE[key] = _build_kernel(zero_mask, zero_bias)
    return _NC_CACHE[key]


def kernel(hidden_states, attention_mask, Wq, bq, Wk, bk, Wv, bv, **run_kw):
    hidden_states = np.asarray(hidden_states, dtype=np.float32)
    attention_mask = np.asarray(attention_mask, dtype=np.float32)
    Wq, Wk, Wv = (np.asarray(a, dtype=np.float32) for a in (Wq, Wk, Wv))
    bq, bk, bv = (np.asarray(a, dtype=np.float32) for a in (bq, bk, bv))

    zero_mask = bool(np.all(attention_mask == 0.0))
    zero_bias = bool(np.all(bq == 0.0) and np.all(bk == 0.0)
                     and np.all(bv == 0.0))
    nc = _get_nc(zero_mask, zero_bias)
    in_maps = []
    for c in range(8):
        b, g = c // 4, c % 4
        cols = slice(192 * g, 192 * (g + 1))
        in_maps.append({
            "x": np.ascontiguousarray(hidden_states[b]),
            "wq": np.ascontiguousarray(Wq[:, cols]),
            "wk": np.ascontiguousarray(Wk[:, cols]),
            "wv": np.ascontiguousarray(Wv[:, cols]),
            "bq": np.ascontiguousarray(bq[cols]),
            "bk": np.ascontiguousarray(bk[cols]),
            "bv": np.ascontiguousarray(bv[cols]),
            "mask": np.ascontiguousarray(
                np.broadcast_to(attention_mask[b, 0, 0], (S,))),
        })
    res = run_bass_kernel_spmd(nc, in_maps, list(range(8)), **run_kw)
    out = np.empty((2, S, DM), dtype=np.float32)
    for c in range(8):
        b, g = c // 4, c % 4
        out[b, :, 192 * g:192 * (g + 1)] = res.results[c]["out"]
    if DEBUG:
        kernel.last_debug = res.results
    if run_kw:
        return out, res
    return out


# revision 41
# speedup vs baseline: 1.3366x; 1.3366x over previous
"""BERT self-attention Bass/Tile kernel for 8 Trainium2 NeuronCores.

Problem: hidden [2, 2048, 768], 12 heads x 64 dim, additive mask [2,1,1,2048].
Sharding: batch x head-group. Core c handles batch b = c // 4 and global heads
3*(c%4) .. 3*(c%4)+2 (columns 192*(c%4) .. +192 of Wq/Wk/Wv).  Each core
computes its 3 heads' full attention locally; outputs are concatenated on the
host (no cross-device communication).

Per-core pipeline (one TileContext):
  X [2048,768] --gpsimd cast fp16, PE transpose--> X_T [768, 2048] (fp16)
  Q_T/K_T = W.T @ X_T  (heads 0,1 packed M=128; head2's Q+K col-tiled M=64)
  V[k, d] computed DIRECTLY via lhsT=X_T chunk (no V transpose):
      V[kc] layout [V_h0|e|V_h1|e|V_h2|e], e = exp(mask_k) (1.0 when mask==0)
  scores_T[k, q] = K_T.T @ Q_T, N=512 per matmul; two streams row-tiled
      (prow 0 / 64) write the two 512-col halves of one [128,1024] PSUM tile
  probs = exp(scores/8) fp16:
      ~75% of tiles on ScalarE (activation Exp, scale=0.125)
      ~25% on VectorE via a Schraudolph bit-trick: i16 = round(A*s + B)
      bitcast to fp16 (~1.8% rms rel err; denominator from the same probs
      cancels the common mode, final rel err ~1e-2 worst case)
  ctx[q, 65] accumulation per (head, q-subtile): 16 matmuls N=65, all three
      heads sharing one PSUM bank (cols 0:65|65:130|130:195)
  normalize: one reciprocal [128,3] + one scalar_tensor_tensor with a
      stride-0 broadcast of the reciprocals -> out tile -> DMA

The additive mask is folded into V: exp(s + m_k) = exp(s) * exp(m_k); both
numerator and denominator columns are pre-scaled by exp(m_k).  Zero-mask and
zero-bias (the common case) compile specialized variants.
"""

import numpy as np

import concourse.bass as bass
import concourse.tile as tile
from concourse import bacc, mybir
from concourse.bass_utils import run_bass_kernel_spmd
from concourse.masks import make_identity

F32 = mybir.dt.float32
F16 = mybir.dt.float16
I16 = mybir.dt.int16
EXP = mybir.ActivationFunctionType.Exp
MULT = mybir.AluOpType.mult
ADD = mybir.AluOpType.add

S = 2048           # sequence length
DM = 768           # model dim
DH = 64            # head dim
FC = DM // 128     # 6 f-chunks (contraction for projections)
KC = S // 128      # 16 k-chunks
QB = 512           # q block width (one N=512 matmul per k-chunk per stream)

# Schraudolph fp16 exp bits: i16 = round(184.66496*s + 15301) ~ fp16(exp(s/8))
SCHR_A = 184.66496
SCHR_B = 15360.0 - 59.0
DVE_EXP = True
DEBUG = False

# Score units: A-units pair heads 0/1 on one q-block J and k-chunk t;
# C-units pair head2 with itself on (kc=2t, kc=2t+1) of q-block J.  Units are
# ordered J-major, interleaved (A 2i, A 2i+1, C i) so that the ctx chains of
# q-block J can stream kc-by-kc right behind score production.


def _build_kernel(zero_mask: bool, zero_bias: bool) -> bass.Bass:
    nc = bacc.Bacc()

    x_d = nc.declare_dram_parameter("x", [S, DM], F32, isOutput=False)
    wq_d = nc.declare_dram_parameter("wq", [DM, 192], F32, isOutput=False)
    wk_d = nc.declare_dram_parameter("wk", [DM, 192], F32, isOutput=False)
    wv_d = nc.declare_dram_parameter("wv", [DM, 192], F32, isOutput=False)
    bq_d = nc.declare_dram_parameter("bq", [192], F32, isOutput=False)
    bk_d = nc.declare_dram_parameter("bk", [192], F32, isOutput=False)
    bv_d = nc.declare_dram_parameter("bv", [192], F32, isOutput=False)
    m_d = nc.declare_dram_parameter("mask", [S], F32, isOutput=False)
    out_d = nc.declare_dram_parameter("out", [S, 192], F32, isOutput=True)
    dbg = {}
    if DEBUG:
        for nm, shape in [("d_xt0", [128, S]), ("d_qt2", [128, S]),
                          ("d_kt2", [128, S]), ("d_qts", [128, S]),
                          ("d_kts", [128, S]), ("d_v0", [128, 195]),
                          ("d_pa", [128, 1024]), ("d_pc", [128, 1024]),
                          ("d_pc_all", [128, 8192]), ("d_v_all", [128, 3120])]:
            dbg[nm] = nc.declare_dram_parameter(nm, shape, F16, isOutput=True)

    with tile.TileContext(nc) as tc:
        _attention(tc, x_d, (wq_d, wk_d, wv_d), (bq_d, bk_d, bv_d), m_d, out_d,
                   zero_mask, zero_bias, dbg)
    nc.compile()
    return nc


def _attention(tc, x_d, w_ds, b_ds, m_d, out_d, zero_mask, zero_bias, dbg={}):
    nc = tc.nc

    def dbg_dump(name, src):
        if name in dbg:
            nc.sync.dma_start(out=dbg[name][:, :], in_=src)

    const = tc.alloc_tile_pool(name="const", bufs=1)
    xpool = tc.alloc_tile_pool(name="xpool", bufs=5)
    persist = tc.alloc_tile_pool(name="persist", bufs=1)
    probs_pool = tc.alloc_tile_pool(name="probs", bufs=44)
    small = tc.alloc_tile_pool(name="small", bufs=4)
    outp = tc.alloc_tile_pool(name="outp", bufs=1)
    ps = tc.alloc_tile_pool(name="ps", bufs=2, space="PSUM")

    # The m=0 X DMAs are issued first so the SP queue starts the big loads
    # before anything else; they overlap all of the setup below.
    x32_tiles = {}

    def load_x_dma(m):
        for j in range(4):
            qt = 4 * m + j
            xt32 = xpool.tile([128, DM], F32, name=f"x_{qt}", tag="x", bufs=8)
            nc.sync.dma_start(out=xt32, in_=x_d[128 * qt:128 * (qt + 1), :])
            x32_tiles[qt] = xt32

    load_x_dma(0)

    # --- constants: mask, identity, weights, biases -------------------------
    mask_t = const.tile([128, KC], F32)  # mask[128*i + p] at [p, i]
    nc.gpsimd.dma_start(out=mask_t, in_=m_d[:].rearrange("(i p) -> p i", p=128))
    expm = const.tile([128, KC], F32)    # exp(mask); also preloads exp table
    nc.scalar.activation(expm, mask_t, EXP)

    ident16 = const.tile([128, 128], F16)
    make_identity(nc, ident16)

    # PE warmup: ~3.5us of junk matmuls flips the HAM clock gate to 2.4 GHz
    # while the first X DMAs land.
    warm = ps.tile([128, 512], F32, name="warm", tag="sc", bufs=2)
    for i in range(30):
        nc.tensor.matmul(warm[:, 0:128], ident16, ident16,
                         start=True, stop=True)

    w16 = {}   # (t, f) -> [128, 192] fp16
    for t in range(3):
        for f in range(FC):
            w32 = small.tile([128, 192], F32, name=f"w32_{t}_{f}", tag="w32",
                             bufs=9)
            nc.gpsimd.dma_start(out=w32, in_=w_ds[t][128 * f:128 * (f + 1), :])
            wt = const.tile([128, 192], F16, name=f"w16_{t}_{f}")
            nc.vector.tensor_copy(out=wt, in_=w32)
            w16[(t, f)] = wt

    bias_pair = []   # [128,1] heads 0,1 of each of q,k
    bias_solo = []   # [128,1] head2, duplicated into both halves
    bvb = None       # [128,192] broadcast of bv (added to the output)
    if not zero_bias:
        for t in range(2):
            bp = const.tile([128, 1], F32, name=f"bias_pair_{t}")
            nc.gpsimd.dma_start(
                out=bp, in_=b_ds[t][0:128].rearrange("(p o) -> p o", o=1))
            bias_pair.append(bp)
            bs = const.tile([128, 1], F32, name=f"bias_solo_{t}")
            nc.gpsimd.dma_start(
                out=bs[0:64], in_=b_ds[t][128:192].rearrange("(p o) -> p o", o=1))
            nc.gpsimd.dma_start(
                out=bs[64:128], in_=b_ds[t][128:192].rearrange("(p o) -> p o", o=1))
            bias_solo.append(bs)
        # bvb[p, d] = bv[d] via a K=1 matmul broadcast: ones[1,128].T @ bv[1,192]
        ones_row = const.tile([1, 128], F32, name="ones_row")
        nc.gpsimd.memset(ones_row, 1.0)
        bv_row = const.tile([1, 192], F32, name="bv_row")
        nc.gpsimd.dma_start(out=bv_row, in_=b_ds[2][:].rearrange("(o d) -> o d", o=1))
        bvp = ps.tile([128, 512], F32, name="bvp", tag="sm", bufs=2)
        nc.tensor.matmul(bvp[:, 0:192], ones_row, bv_row, start=True, stop=True)
        bvb = const.tile([128, 192], F32, name="bvb")
        nc.vector.tensor_copy(out=bvb, in_=bvp[:, 0:192])

    # --- persistent projection outputs --------------------------------------
    XT = [persist.tile([128, S], F16, name=f"XT_{f}") for f in range(FC)]
    QT2 = persist.tile([128, S], F16)   # rows 0:64 head0, 64:128 head1
    KT2 = persist.tile([128, S], F16)
    QTs = persist.tile([128, S], F16)   # head2 duplicated into both halves
    KTs = persist.tile([128, S], F16)
    # V[kc] layout: [V_h0(64) | e | V_h1(64) | e | V_h2(64) | e]
    V = [persist.tile([128, 195], F16, name=f"V_{kc}") for kc in range(KC)]

    out_tiles = [outp.tile([128, 192], F32, name=f"o_{u}") for u in range(16)]

    def load_x_block(m):
        """Cast fp16 on gpsimd, PE-transpose into XT."""
        x16s = []
        for j in range(4):
            qt = 4 * m + j
            xt32 = x32_tiles.pop(qt)
            x16 = xpool.tile([128, DM], F16, name=f"x16_{qt}", tag="x16")
            nc.vector.tensor_copy(out=x16, in_=xt32)
            x16s.append(x16)
        for f in range(FC):
            tp = ps.tile([128, 512], F16, name=f"xt_ps_{m}_{f}", tag="sm",
                         bufs=2)
            for j in range(4):
                nc.tensor.transpose(
                    tp[:, 128 * j:128 * (j + 1)],
                    x16s[j][:, 128 * f:128 * (f + 1)],
                    ident16,
                )
            nc.vector.tensor_copy(out=XT[f][:, 512 * m:512 * (m + 1)], in_=tp)

    def proj_pair(t, dst_pair, m):
        """Heads 0,1 of q (t=0) or k (t=1): M=128 weight, N=512 stream."""
        cols = slice(512 * m, 512 * (m + 1))
        pp = ps.tile([128, 512], F32, name=f"proj_{t}_{m}_p", tag="sm", bufs=2)
        for f in range(FC):
            nc.tensor.matmul(pp, w16[(t, f)][:, 0:128], XT[f][:, cols],
                             start=(f == 0), stop=(f == FC - 1))
        if zero_bias:
            nc.vector.tensor_copy(out=dst_pair[:, cols], in_=pp)
        else:
            nc.vector.tensor_scalar_add(out=dst_pair[:, cols], in0=pp,
                                        scalar1=bias_pair[t])

    def proj_qk2(m):
        """Head2's Q and K, col-tiled M=64 chains sharing one PSUM bank.

        Q lands in rows qrow:qrow+64 (qrow = 0 for even m, 64 for odd m) so
        QTs gets each q-block at the row parity the paired score streams
        expect; K lands in the other half and is duplicated into both halves
        of KTs (and QTs' other half) by SBUF->SBUF DMA."""
        cols = slice(512 * m, 512 * (m + 1))
        qrow = 0 if m % 2 == 0 else 64
        krow = 64 - qrow
        sp = ps.tile([128, 512], F32, name=f"proj_s_{m}", tag="sm", bufs=2)
        prev = None
        for f in range(FC):
            ma = nc.tensor.matmul(sp[qrow:qrow + 64], w16[(0, f)][:, 128:192],
                                  XT[f][:, cols],
                                  start=(f == 0), stop=(f == FC - 1))
            if prev is not None:
                tile.add_dep_helper(ma.ins, prev.ins, sync=False,
                                    reason="qk2 col-pair order")
            mb = nc.tensor.matmul(sp[krow:krow + 64], w16[(1, f)][:, 128:192],
                                  XT[f][:, cols],
                                  start=(f == 0), stop=(f == FC - 1))
            tile.add_dep_helper(mb.ins, ma.ins, sync=False,
                                reason="qk2 col-pair order")
            prev = mb
        if zero_bias:
            nc.vector.tensor_copy(out=QTs[qrow:qrow + 64, cols],
                                  in_=sp[qrow:qrow + 64])
            nc.vector.tensor_copy(out=KTs[krow:krow + 64, cols],
                                  in_=sp[krow:krow + 64])
        else:
            nc.vector.tensor_scalar_add(out=QTs[qrow:qrow + 64, cols],
                                        in0=sp[qrow:qrow + 64],
                                        scalar1=bias_solo[0][qrow:qrow + 64])
            nc.vector.tensor_scalar_add(out=KTs[krow:krow + 64, cols],
                                        in0=sp[krow:krow + 64],
                                        scalar1=bias_solo[1][krow:krow + 64])
        # duplicate head2 q/k into the other partition half for row tiling
        nc.gpsimd.dma_start(out=QTs[krow:krow + 64, cols],
                            in_=QTs[qrow:qrow + 64, cols])
        nc.gpsimd.dma_start(out=KTs[qrow:qrow + 64, cols],
                            in_=KTs[krow:krow + 64, cols])

    def v_direct(kc):
        """V[kc][k, d] = X[k,:] @ Wv directly: lhsT = X_T chunk (FWL'd fp16
        weights), rhs = Wv chunk, N=192.  No transpose, one strided copy."""
        vp = ps.tile([128, 512], F32, name=f"v_ps_{kc}", tag="sm", bufs=2)
        for f in range(FC):
            nc.tensor.matmul(vp[:, 0:192],
                             XT[f][:, 128 * kc:128 * (kc + 1)],
                             w16[(2, f)],
                             start=(f == 0), stop=(f == FC - 1))
        vdst = bass.AP(tensor=V[kc].tensor, offset=V[kc].offset,
                       ap=[V[kc].ap[0], [65, 3], [1, 64]])
        vsrc = bass.AP(tensor=vp.tensor, offset=vp.offset,
                       ap=[vp.ap[0], [64, 3], [1, 64]])
        ecol = bass.AP(tensor=V[kc].tensor, offset=V[kc].offset + 64,
                       ap=[V[kc].ap[0], [65, 3]])
        if zero_mask and zero_bias:
            nc.vector.tensor_copy(out=vdst, in_=vsrc)
        elif zero_mask:
            # bias folds into the output (ctx = ctx_nobias + bv); plain copy
            nc.vector.tensor_copy(out=vdst, in_=vsrc)
        else:
            # (v [+ bv]) * exp(m_k); bv also shifts ctx but the mask-weighted
            # denominator no longer cancels it, so scale V directly.
            tmp = small.tile([128, 192], F32, name=f"vt_{kc}", tag="vtmp")
            if zero_bias:
                nc.vector.tensor_copy(out=tmp, in_=vp[:, 0:192])
            else:
                nc.vector.tensor_tensor(out=tmp, in0=vp[:, 0:192], in1=bvb,
                                        op=ADD)
            tsrc = bass.AP(tensor=tmp.tensor, offset=tmp.offset,
                           ap=[tmp.ap[0], [64, 3], [1, 64]])
            nc.vector.tensor_scalar_mul(out=vdst, in0=tsrc,
                                        scalar1=expm[:, kc:kc + 1])
        if zero_mask:
            nc.gpsimd.memset(ecol, 1.0)
        else:
            esrc = bass.AP(tensor=expm.tensor, offset=expm.offset + kc,
                           ap=[expm.ap[0], [0, 3]])
            nc.vector.tensor_copy(out=ecol, in_=esrc)

    # --- scores + exp --------------------------------------------------------
    # unit = (kind, J, t): one [128,1024] PSUM tile, two N=512 matmuls on
    # disjoint row halves, one exp (ScalarE or VectorE-Schraudolph).
    # A-unit: streams (h0,J,prow0)+(h1,J,prow64), both on kc=t.
    # C-unit: stream (h2,J,prow0) on kc=2t, (h2,J,prow64) on 2t+1.
    probs_store = {}   # (head, J, kc) -> (tile, col_offset)

    def emit_unit(kind, J, t):
        sc = ps.tile([128, 1024], F32, name=f"sc_{kind}_{J}_{t}", tag="sc",
                     bufs=2)
        if kind == "A":
            streams = [(0, J, 0, t), (1, J, 64, t)]
        else:
            streams = [(2, J, 0, 2 * t), (2, J, 64, 2 * t + 1)]
        prev = None
        for i, (h, Jq, prow, kc) in enumerate(streams):
            KT = KT2 if h < 2 else KTs
            QT = QT2 if h < 2 else QTs
            mm = nc.tensor.matmul(
                sc[:, 512 * i:512 * (i + 1)],
                KT[prow:prow + 64, 128 * kc:128 * (kc + 1)],
                QT[prow:prow + 64, 512 * Jq:512 * (Jq + 1)],
                start=True, stop=True)
            if prev is not None:
                tile.add_dep_helper(mm.ins, prev.ins, sync=False,
                                    reason="score pair adjacency")
            prev = mm
        pt = probs_pool.tile([128, 1024], F16, name=f"pb_{kind}_{J}_{t}",
                             tag="probs")
        if DVE_EXP and t % 4 == 1:
            # VectorE Schraudolph: i16 bits of fp16(exp(s/8))
            nc.vector.tensor_scalar(
                pt[:, 0:1024].bitcast(I16), sc[:, 0:1024],
                SCHR_A, SCHR_B, MULT, ADD)
        else:
            nc.scalar.activation(pt, sc, EXP, scale=0.125)
        for i, (h, Jq, prow, kc) in enumerate(streams):
            probs_store[(h, Jq, kc)] = (pt, 512 * i)
        if kind == "A":
            a_cnt[J] += 1
        else:
            c_cnt[J] += 1
        if J == 0 and t == 0:
            dbg_dump("d_pa" if kind == "A" else "d_pc", pt)
        if "d_pc_all" in dbg and kind == "C" and J == 0:
            nc.sync.dma_start(out=dbg["d_pc_all"][:, 1024 * t:1024 * (t + 1)],
                              in_=pt)

    # --- ctx chains ----------------------------------------------------------
    # The 4 q-subtiles of q-block J accumulate concurrently, two per PSUM
    # bank: cxA holds u=4J+0 (cols 0:195) and u=4J+1 (cols 256:451), cxB the
    # other two.  Only the head J-block paces (strict FIFO) so the two cx
    # ring slots can never be claimed out of order.  Chain matmuls stream
    # kc-by-kc as soon as all three heads' probs for that kc exist.
    a_cnt = [0] * 4
    c_cnt = [0] * 4
    v_cnt = [0]        # v_direct(kc) emitted for kc < v_cnt[0]

    class JBlock:
        def __init__(self, J):
            self.J = J
            self.kc = 0
            self.cx = None     # [cxA, cxB]
            self.prev = [None, None]
            self.done = False

        def avail(self):
            return min(a_cnt[self.J], 2 * c_cnt[self.J], v_cnt[0])

        def pace(self):
            J = self.J
            hi = self.avail()
            if hi <= self.kc:
                return
            if self.cx is None:
                self.cx = [
                    ps.tile([128, 512], F32, name=f"cxA_{J}", tag="cx",
                            bufs=2),
                    ps.tile([128, 512], F32, name=f"cxB_{J}", tag="cx",
                            bufs=2),
                ]
            for kc in range(self.kc, hi):
                for pair in range(2):
                    cx = self.cx[pair]
                    for du in range(2):
                        s = 2 * pair + du
                        base = 256 * du
                        for h in range(3):
                            pt, off = probs_store[(h, J, kc)]
                            mm = nc.tensor.matmul(
                                cx[:, base + 65 * h:base + 65 * h + 65],
                                pt[:, off + 128 * s:off + 128 * (s + 1)],
                                V[kc][:, 65 * h:65 * h + 65],
                                start=(kc == 0 and du == 0 and h == 0),
                                stop=(kc == KC - 1))
                            if self.prev[pair] is not None:
                                tile.add_dep_helper(
                                    mm.ins, self.prev[pair].ins, sync=False,
                                    reason="cx chain order")
                            self.prev[pair] = mm
            self.kc = hi
            if self.kc == KC:
                for pair in range(2):
                    for du in range(2):
                        self._finish_u(4 * self.J + 2 * pair + du,
                                       self.cx[pair], 256 * du)
                self.done = True

        def _finish_u(self, u, cx, base):
            r = small.tile([128, 3], F32, name=f"r_{u}", tag="recip")
            den = bass.AP(tensor=cx.tensor, offset=cx.offset + base + 64,
                          ap=[cx.ap[0], [65, 3]])
            nc.vector.reciprocal(r, den)
            for h in range(3):
                nc.vector.tensor_scalar_mul(
                    out=out_tiles[u][:, 64 * h:64 * (h + 1)],
                    in0=cx[:, base + 65 * h:base + 65 * h + 64],
                    scalar1=r[:, h:h + 1])
            if not zero_bias and zero_mask:
                nc.vector.tensor_tensor(out=out_tiles[u], in0=out_tiles[u],
                                        in1=bvb, op=ADD)
            nc.sync.dma_start(out=out_d[128 * u:128 * (u + 1), :],
                              in_=out_tiles[u])

    # --- emission ------------------------------------------------------------
    units = []
    for J in range(4):
        for i in range(8):
            units.append(("A", J, 2 * i))
            units.append(("A", J, 2 * i + 1))
            units.append(("C", J, i))
    emitted = set()
    jqueue = [JBlock(J) for J in range(4)]

    def pace():
        while jqueue:
            jb = jqueue[0]
            jb.pace()
            if jb.done:
                jqueue.pop(0)
                continue
            break

    def try_emit(q_m, k_m, c_m, budget):
        # A-units: QT2 q-block J from proj_pair(Q, J), KT2 kc from
        # proj_pair(K, kc//4).  C-units: QTs/KTs both come from proj_qk2.
        for (kind, J, t) in units:
            if budget <= 0:
                return
            if (kind, J, t) in emitted:
                continue
            kmax = t if kind == "A" else 2 * t + 1
            if kind == "A":
                ok = J <= q_m and kmax // 4 <= k_m
            else:
                ok = J <= c_m and kmax // 4 <= c_m
            if ok:
                emit_unit(kind, J, t)
                emitted.add((kind, J, t))
                pace()
                budget -= 1

    for m in range(4):
        if m < 3:
            load_x_dma(m + 1)
        load_x_block(m)
        proj_pair(1, KT2, m)   # K first: scores need K columns
        try_emit(m - 1, m - 1, m - 1, 3)
        proj_pair(0, QT2, m)
        try_emit(m - 1, m, m - 1, 3)
        proj_qk2(m)
        try_emit(m, m, m, 4)
        for kc in range(4 * m, 4 * m + 4):
            v_direct(kc)
            v_cnt[0] = kc + 1
            try_emit(m, m, m, 2)

    if "d_v_all" in dbg:
        for kc in range(KC):
            nc.sync.dma_start(out=dbg["d_v_all"][:, 195 * kc:195 * (kc + 1)],
                              in_=V[kc])
    dbg_dump("d_xt0", XT[0])
    dbg_dump("d_qt2", QT2)
    dbg_dump("d_kt2", KT2)
    dbg_dump("d_qts", QTs)
    dbg_dump("d_kts", KTs)
    dbg_dump("d_v0", V[0])

    # steady state: remaining units interleaved with ctx chains
    for (kind, J, t) in units:
        if (kind, J, t) in emitted:
            continue
        emit_unit(kind, J, t)
        emitted.add((kind, J, t))
        pace()
    pace()

    for p in (ps, outp, small, probs_pool, persist, xpool, const):
        p.release()


_NC_CACHE = {}


def _get_nc(zero_mask: bool, zero_bias: bool):
    key = (zero_mask, zero_bias)
    if key not in _NC_CACHE:
        _NC_CACHE[key] = _build_kernel(zero_mask, zero_bias)
    return _NC_CACHE[key]


def kernel(hidden_states, attention_mask, Wq, bq, Wk, bk, Wv, bv, **run_kw):
    hidden_states = np.asarray(hidden_states, dtype=np.float32)
    attention_mask = np.asarray(attention_mask, dtype=np.float32)
    Wq, Wk, Wv = (np.asarray(a, dtype=np.float32) for a in (Wq, Wk, Wv))
    bq, bk, bv = (np.asarray(a, dtype=np.float32) for a in (bq, bk, bv))

    zero_mask = bool(np.all(attention_mask == 0.0))
    zero_bias = bool(np.all(bq == 0.0) and np.all(bk == 0.0)
                     and np.all(bv == 0.0))
    nc = _get_nc(zero_mask, zero_bias)
    in_maps = []
    for c in range(8):
        b, g = c // 4, c % 4
        cols = slice(192 * g, 192 * (g + 1))
        in_maps.append({
            "x": np.ascontiguousarray(hidden_states[b]),
            "wq": np.ascontiguousarray(Wq[:, cols]),
            "wk": np.ascontiguousarray(Wk[:, cols]),
            "wv": np.ascontiguousarray(Wv[:, cols]),
            "bq": np.ascontiguousarray(bq[cols]),
            "bk": np.ascontiguousarray(bk[cols]),
            "bv": np.ascontiguousarray(bv[cols]),
            "mask": np.ascontiguousarray(
                np.broadcast_to(attention_mask[b, 0, 0], (S,))),
        })
    res = run_bass_kernel_spmd(nc, in_maps, list(range(8)), **run_kw)
    out = np.empty((2, S, DM), dtype=np.float32)
    for c in range(8):
        b, g = c // 4, c % 4
        out[b, :, 192 * g:192 * (g + 1)] = res.results[c]["out"]
    if DEBUG:
        kernel.last_debug = res.results
    if run_kw:
        return out, res
    return out
